# revision 39
# baseline (speedup 1.0000x reference)
"""Tensor-parallel MultiHeadAttention (LN + fused QKV + causal SDPA + proj)
for 8 Trainium2 NeuronCores.

Sharding: 2 heads per core. LayerNorm gamma/beta folded into qkv weights on
host; LN (x-mu)*rstd applied via rank-1 PSUM corrections + evacuation scaling.
QKV/scores matmuls run in fp32r; softmax probs, V, and the context are bf16.

Collective strategy: the CC transport is element-rate-bound (~17 G elem/s
regardless of dtype or reduce-op), so instead of ReduceScattering dense
[512,2048] proj partials (1M elements per s-block), each core AllToAlls its
rank-compressed context slices (128K elements per s-block: its 256 hidden
dims x each destination's 64 output rows). Every core then computes the FULL
output projection for its own 256 output rows against resident bf16 proj
weights and writes f32 output directly - no reduction collective at all.
Host reassembles the full [S,1,HID] output.
"""

import sys

sys.path.insert(0, "/opt/trn_rl_repo")

import math

import numpy as np

S, HID, NH, HD = 2048, 2048, 16, 128
EPS = 1e-5
NCORES = 8
HPC = NH // NCORES        # heads per core: 2
OQK = 2 * HPC * HD        # q+k rows per core: 512
OV = HPC * HD             # v rows per core: 256
KO = HID // 128           # contraction chunks: 16
NSB = S // 512            # s-blocks: 4
NTB = S // 128            # t-blocks: 16
RS_OUT = 512 // NCORES    # output rows per core per s-block: 64
SCALE = 1.0 / math.sqrt(HD)
MASKVAL = -30000.0

_CACHE = {}


def _build_nc(debug=False, sim_mode=False):
    import concourse.mybir as mybir
    import concourse.tile as tile
    from concourse import bacc
    from contextlib import ExitStack

    f32 = mybir.dt.float32
    f32r = mybir.dt.float32r
    bf16 = mybir.dt.bfloat16
    Act = mybir.ActivationFunctionType

    nc = bacc.Bacc(num_devices=NCORES)

    # ---- I/O ----
    xT_d = nc.dram_tensor("xT", [HID, S], f32r, kind="ExternalInput")
    wqkT_d = nc.dram_tensor("wqkT", [HID, OQK], f32r, kind="ExternalInput")
    wvT_d = nc.dram_tensor("wvT", [HID, OV], f32r, kind="ExternalInput")
    wpf_d = nc.dram_tensor("wpf", [HID, HID], bf16, kind="ExternalInput")
    rsum_qk_d = nc.dram_tensor("rsum_qk", [1, OQK], f32r, kind="ExternalInput")
    rsum_v_d = nc.dram_tensor("rsum_v", [1, OV], f32r, kind="ExternalInput")
    bqk_d = nc.dram_tensor("bqk", [1, OQK], f32r, kind="ExternalInput")
    bv_d = nc.dram_tensor("bv", [1, OV], f32r, kind="ExternalInput")
    pbias_d = nc.dram_tensor("pbias", [1, HID], bf16, kind="ExternalInput")
    maskneg_d = nc.dram_tensor("maskneg", [128, 128], bf16, kind="ExternalInput")
    ident_d = nc.dram_tensor("ident", [128, 128], bf16, kind="ExternalInput")
    ones_d = nc.dram_tensor("ones_col", [128, 1], f32r, kind="ExternalInput")
    onesb_d = nc.dram_tensor("ones_bf", [128, 1], bf16, kind="ExternalInput")
    onesrow_d = nc.dram_tensor("ones_row_bf", [1, 128], bf16, kind="ExternalInput")
    out_d = nc.dram_tensor("out", [NSB * RS_OUT, HID], f32, kind="ExternalOutput")

    # internal DRAM: stats round trips + ctx exchange
    rstd_dram = nc.dram_tensor("rstd_scratch", [NSB, 512], f32)
    ctx_send = nc.dram_tensor(
        "ctx_send", [NSB, HPC, NCORES, 128, RS_OUT], bf16
    )
    ctx_recv = nc.dram_tensor(
        "ctx_recv", [NSB, HPC, NCORES, 128, RS_OUT], bf16
    )
    warm_s = nc.dram_tensor("warm_s", [NCORES, 64], bf16)
    warm_r = nc.dram_tensor("warm_r", [NCORES, 64], bf16)

    ctx = ExitStack()
    with ctx:
        tc = ctx.enter_context(tile.TileContext(nc))
        # resident pools (whole kernel lifetime)
        wpool = ctx.enter_context(tc.tile_pool(name="wpool", bufs=1))
        rows = ctx.enter_context(tc.tile_pool(name="rows", bufs=1))
        bigout = ctx.enter_context(tc.tile_pool(name="bigout", bufs=1))
        statrow = ctx.enter_context(tc.tile_pool(name="statrow", bufs=1))

        # ---- small resident rows ----
        ones_col = rows.tile([128, 1], f32r)
        nc.sync.dma_start(out=ones_col, in_=ones_d[:, :])
        ones_bf = rows.tile([128, 1], bf16)
        nc.sync.dma_start(out=ones_bf, in_=onesb_d[:, :])
        ones_row = rows.tile([1, 128], bf16)
        nc.sync.dma_start(out=ones_row, in_=onesrow_d[:, :])
        pbias_row = rows.tile([1, HID], bf16)
        nc.sync.dma_start(out=pbias_row, in_=pbias_d[:, :])
        rsum_qk = rows.tile([1, OQK], f32r)
        nc.sync.dma_start(out=rsum_qk, in_=rsum_qk_d[:, :])
        rsum_v = rows.tile([1, OV], f32r)
        nc.sync.dma_start(out=rsum_v, in_=rsum_v_d[:, :])
        bqk = rows.tile([1, OQK], f32r)
        nc.sync.dma_start(out=bqk, in_=bqk_d[:, :])
        bv = rows.tile([1, OV], f32r)
        nc.sync.dma_start(out=bv, in_=bv_d[:, :])
        eps_tile = rows.tile([128, 1], f32)
        nc.vector.memset(eps_tile, EPS)
        maskneg = rows.tile([128, 128], bf16)
        ident = rows.tile([128, 128], bf16)
        wpf = wpool.tile([128, KO, HID], bf16)

        # ---- persistent phase-1 outputs ----
        kT = [bigout.tile([128, S], f32r, name=f"kT{h}") for h in range(HPC)]
        vtile = bigout.tile([128, NTB, OV], bf16, name="vtile")
        ctxT = [bigout.tile([128, S], bf16, name=f"ctxT{h}") for h in range(HPC)]
        rstd_col = bigout.tile([128, NSB * 4], f32, name="rstd_col")
        ctxall = [
            bigout.tile([128, HPC, NCORES, 128], bf16, name=f"ctxall{p}")
            for p in range(2)
        ]

        # =========================================================
        # Per-sb pipeline: phase1(sb) -> attention(sb) -> ctx A2A(sb).
        # Final proj runs per sb-pair once that pair's A2A has landed.
        # =========================================================
        with (
            tc.tile_pool(name="wqkv", bufs=1) as wqkv,
            tc.tile_pool(name="xpool", bufs=2) as xpool,
            tc.tile_pool(name="qcur", bufs=2) as qpool,
            tc.tile_pool(name="sqpool", bufs=2) as sqpool,
            tc.tile_pool(name="rowr", bufs=1) as rowr,
            tc.tile_pool(name="bcast", bufs=1) as bcastp,
            tc.tile_pool(name="exppool", bufs=4) as exppool,
            tc.tile_pool(name="outpool", bufs=2) as outpool,
            tc.tile_pool(name="ps", bufs=8, space="PSUM") as psp,
        ):
            wqkT = wqkv.tile([128, KO, OQK], f32r)
            wvT = wqkv.tile([128, KO, OV], f32r)
            nc.sync.dma_start(out=maskneg, in_=maskneg_d[:, :])
            nc.sync.dma_start(out=ident, in_=ident_d[:, :])

            # warm-up collective: absorbs the runtime barrier, core-skew and
            # first-op ramp during the input-DMA window, so the first real
            # ctx exchange runs at steady-state cost.
            if not sim_mode:
                wtile = rows.tile([NCORES, 64], bf16, name="wtile")
                nc.vector.memset(wtile, 0.0)
                nc.sync.dma_start(out=warm_s[:, :], in_=wtile)
                nc.gpsimd.collective_compute(
                    "AllToAll",
                    mybir.AluOpType.bypass,
                    replica_groups=[list(range(NCORES))],
                    ins=[warm_s.ap()],
                    outs=[warm_r.ap()],
                )
                # read the warm-up result back on the gpsimd queue: hard-
                # orders the first real exchange after warm-up completion
                nc.gpsimd.dma_start(out=wtile, in_=warm_r[:, :])

            def emit_pair_loads(p, heads=(0, 1)):
                """SBUF loads of the gathered ctx^T for pair p, on the sync
                queue at a program point where the A2As have completed."""
                for j in range(2):
                    sbx = 2 * p + j
                    for h in heads:
                        nc.sync.dma_start(
                            out=ctxall[p][:, h, :, j * 64 : (j + 1) * 64],
                            in_=ctx_recv[sbx, h].rearrange("c p r -> p c r"),
                        )

            def emit_pair_proj(p, head_major=False):
                """Full-depth output projection for rows of s-blocks
                (2p, 2p+1): lhsT = gathered ctx^T [128 hid-chunk, 128 rows],
                rhs = resident bf16 proj weights; bias via rank-1 matmul.
                head_major runs all heads' h0 chunks first across 4 PSUM
                banks so the tail pair only waits on the last head's A2A."""
                if head_major:
                    ps_os = [
                        psp.tile([128, 512], f32, tag="bank", name=f"ps_o{p}_{ob}")
                        for ob in range(4)
                    ]
                    for h in range(HPC):
                        for ob in range(4):
                            o0 = ob * 512
                            for c in range(NCORES):
                                nc.tensor.matmul(
                                    ps_os[ob],
                                    ctxall[p][:, h, c, :],
                                    wpf[:, c * HPC + h, o0 : o0 + 512],
                                    start=(h == 0 and c == 0),
                                    stop=False,
                                )
                    for ob in range(4):
                        o0 = ob * 512
                        nc.tensor.matmul(
                            ps_os[ob],
                            ones_row,
                            pbias_row[0:1, o0 : o0 + 512],
                            start=False,
                            stop=True,
                        )
                        otile = outpool.tile([128, 512], f32, tag="otile")
                        nc.vector.tensor_copy(out=otile, in_=ps_os[ob])
                        nc.sync.dma_start(
                            out=out_d[p * 128 : (p + 1) * 128, o0 : o0 + 512],
                            in_=otile,
                        )
                else:
                    for ob in range(4):
                        o0 = ob * 512
                        ps_o = psp.tile(
                            [128, 512], f32, tag="bank", name=f"ps_o{p}_{ob}"
                        )
                        for h in range(HPC):
                            for c in range(NCORES):
                                nc.tensor.matmul(
                                    ps_o,
                                    ctxall[p][:, h, c, :],
                                    wpf[:, c * HPC + h, o0 : o0 + 512],
                                    start=(h == 0 and c == 0),
                                    stop=False,
                                )
                        nc.tensor.matmul(
                            ps_o,
                            ones_row,
                            pbias_row[0:1, o0 : o0 + 512],
                            start=False,
                            stop=True,
                        )
                        otile = outpool.tile([128, 512], f32, tag="otile")
                        nc.vector.tensor_copy(out=otile, in_=ps_o)
                        nc.sync.dma_start(
                            out=out_d[p * 128 : (p + 1) * 128, o0 : o0 + 512],
                            in_=otile,
                        )

            for sb in range(NSB):
                s0 = sb * 512
                # ---------------- phase 1: stats + q/k/v ----------------
                ps_sums = psp.tile([1, 512], f32, tag="bank", name="ps_sums")
                ps_sumsq = psp.tile([1, 512], f32, tag="bank", name="ps_sumsq")
                ps_qk = [
                    psp.tile([128, 512], f32, tag="bank", name=f"ps_qk{ob}")
                    for ob in range(4)
                ]
                # two banks, each packing two 256-wide v accumulation groups
                ps_v = [
                    psp.tile([128, 512], f32, tag="bank", name=f"ps_v{i}")
                    for i in range(2)
                ]
                for hq in range(8):
                    xt2 = xpool.tile([128, 2, 512], f32r, tag="xt",
                                     name=f"xt{sb}_{hq}")
                    nc.sync.dma_start(
                        out=xt2,
                        in_=xT_d[hq * 256 : (hq + 1) * 256, s0 : s0 + 512].rearrange(
                            "(c p) s -> p c s", p=128
                        ),
                    )
                    if sb == 0:
                        nc.sync.dma_start(
                            out=wqkT[:, hq * 2 : (hq + 1) * 2, :],
                            in_=wqkT_d[hq * 256 : (hq + 1) * 256, :].rearrange(
                                "(c p) o -> p c o", p=128
                            ),
                        )
                        if hq % 2 == 0:
                            nc.sync.dma_start(
                                out=wvT[:, hq * 2 : (hq + 2) * 2, :],
                                in_=wvT_d[hq * 256 : (hq + 2) * 256, :].rearrange(
                                    "(c p) o -> p c o", p=128
                                ),
                            )
                    for hh in range(2):
                        h = hq * 2 + hh
                        xt = xt2[:, hh, :]
                        xsq = sqpool.tile([128, 512], f32r, tag="xsq")
                        if h % 2 == 0:
                            nc.scalar.activation(out=xsq, in_=xt, func=Act.Square)
                        else:
                            nc.vector.tensor_mul(out=xsq, in0=xt, in1=xt)
                        nc.tensor.matmul(
                            ps_sums, ones_col, xt, start=(h == 0), stop=(h == KO - 1)
                        )
                        nc.tensor.matmul(
                            ps_sumsq, ones_col, xsq, start=(h == 0),
                            stop=(h == KO - 1)
                        )
                        for ob in range(4):
                            nc.tensor.matmul(
                                ps_qk[ob],
                                wqkT[:, h, ob * 128 : (ob + 1) * 128],
                                xt,
                                start=(h == 0),
                                stop=False,
                            )
                        for vs in range(4):
                            nc.tensor.matmul(
                                ps_v[vs // 2][:, (vs % 2) * 256 : (vs % 2 + 1) * 256],
                                xt[:, vs * 128 : (vs + 1) * 128],
                                wvT[:, h, :],
                                start=(h == 0 and vs % 2 == 0),
                                stop=False,
                                skip_group_check=(vs % 2 == 1),
                            )
                if sb < 2:
                    # proj weights in 2.1MB chunks during the attention
                    # phases of sb0/sb1 (quiet DMA windows; needed ~250us)
                    for k in range(2):
                        cc0 = (sb * 2 + k) * 4
                        nc.sync.dma_start(
                            out=wpf[:, cc0 : cc0 + 4, :],
                            in_=wpf_d[cc0 * 128 : (cc0 + 4) * 128, :].rearrange(
                                "(c p) o -> p c o", p=128
                            ),
                        )

                # stats rows (short critical chain)
                negmu_r = rowr.tile([1, 512], f32r, tag="negmu_r")
                nc.vector.tensor_scalar_mul(
                    out=negmu_r, in0=ps_sums, scalar1=-1.0 / HID
                )
                mu = statrow.tile([1, 512], f32, tag="mu")
                nc.vector.tensor_scalar_mul(out=mu, in0=ps_sums, scalar1=1.0 / HID)
                nc.vector.tensor_mul(out=mu, in0=mu, in1=mu)  # mu := mu^2
                var = statrow.tile([1, 512], f32, tag="var")
                nc.vector.scalar_tensor_tensor(
                    out=var,
                    in0=ps_sumsq,
                    scalar=1.0 / HID,
                    in1=mu,
                    op0=mybir.AluOpType.mult,
                    op1=mybir.AluOpType.subtract,
                )
                invrstd_r = rowr.tile([1, 512], f32r, tag="invrstd_r")
                nc.scalar.activation(
                    out=invrstd_r, in_=var, func=Act.Sqrt, bias=eps_tile[0:1]
                )
                rstd = statrow.tile([1, 512], f32, tag="rstd")
                nc.vector.reciprocal_approx_fast(
                    out=rstd, in_=invrstd_r.bitcast(f32)
                )

                # rstd column layout (DRAM bounce) + partition broadcast
                nc.sync.dma_start(out=rstd_dram[sb : sb + 1, :], in_=rstd)
                nc.sync.dma_start(
                    out=rstd_col[:, sb * 4 : (sb + 1) * 4],
                    in_=rstd_dram[sb, :].rearrange("(f p) -> p f", p=128),
                )
                rstd_b = bcastp.tile([128, 512], f32, tag="rstd_b")
                nc.gpsimd.partition_broadcast(rstd_b, rstd)

                # q/k rank-1 corrections + evac (q transient, k persistent)
                qcur = [
                    qpool.tile([128, 512], f32r, tag=f"q{h}", name=f"q{sb}_{h}")
                    for h in range(HPC)
                ]
                for ob in range(4):
                    nc.tensor.matmul(
                        ps_qk[ob],
                        rsum_qk[0:1, ob * 128 : (ob + 1) * 128],
                        negmu_r,
                        start=False,
                        stop=False,
                    )
                    nc.tensor.matmul(
                        ps_qk[ob],
                        bqk[0:1, ob * 128 : (ob + 1) * 128],
                        invrstd_r,
                        start=False,
                        stop=True,
                    )
                    if ob < 2:
                        nc.vector.tensor_mul(
                            out=qcur[ob], in0=ps_qk[ob], in1=rstd_b
                        )
                    else:
                        nc.vector.tensor_mul(
                            out=kT[ob - 2][:, s0 : s0 + 512],
                            in0=ps_qk[ob],
                            in1=rstd_b,
                        )

                # v rank-1 corrections + evac (bf16 out)
                for vs in range(4):
                    pv = ps_v[vs // 2][:, (vs % 2) * 256 : (vs % 2 + 1) * 256]
                    nc.tensor.matmul(
                        pv,
                        negmu_r[0:1, vs * 128 : (vs + 1) * 128],
                        rsum_v,
                        start=False,
                        stop=False,
                        skip_group_check=True,
                    )
                    nc.tensor.matmul(
                        pv,
                        invrstd_r[0:1, vs * 128 : (vs + 1) * 128],
                        bv,
                        start=False,
                        stop=True,
                        skip_group_check=True,
                    )
                    nc.vector.tensor_scalar_mul(
                        out=vtile[:, sb * 4 + vs, :],
                        in0=pv,
                        scalar1=rstd_col[:, sb * 4 + vs : sb * 4 + vs + 1],
                    )

                # ---------------- attention for this sb ----------------
                ntb = 4 * (sb + 1)  # causal t-blocks
                for h in range(HPC):
                    ps_ctx = psp.tile(
                        [128, 512], f32, tag="bank", name=f"ps_ctx{sb}_{h}"
                    )
                    ps_den = psp.tile(
                        [1, 512], f32, tag="bank", name=f"ps_den{sb}_{h}"
                    )
                    for tb in range(ntb):
                        t0 = tb * 128
                        delta = max(0, t0 - s0)
                        ps_sc = psp.tile([128, 512], f32, tag="bank", name="ps_sc")
                        nc.tensor.matmul(
                            ps_sc[:, delta:512],
                            kT[h][:, t0 : t0 + 128],
                            qcur[h][:, delta:512],
                            start=True,
                            stop=(t0 < s0),
                        )
                        if t0 >= s0:
                            nc.tensor.matmul(
                                ps_sc[:, delta : delta + 128],
                                maskneg,
                                ident,
                                start=False,
                                stop=True,
                            )
                        expt = exppool.tile([128, 512], bf16, tag="expt")
                        nc.scalar.activation(
                            out=expt[:, delta:512],
                            in_=ps_sc[:, delta:512],
                            func=Act.Exp,
                            scale=SCALE,
                        )
                        # columns [0, delta) are invalid (t > s) and never
                        # written: every column's first accumulant is tb==0.
                        nc.tensor.matmul(
                            ps_ctx[:, delta:512],
                            vtile[:, tb, h * HD : (h + 1) * HD],
                            expt[:, delta:512],
                            start=(tb == 0),
                            stop=(tb == ntb - 1),
                            skip_group_check=True,
                        )
                        nc.tensor.matmul(
                            ps_den[:, delta:512],
                            ones_bf,
                            expt[:, delta:512],
                            start=(tb == 0),
                            stop=(tb == ntb - 1),
                            skip_group_check=True,
                        )
                    rden = statrow.tile([1, 512], f32, tag="rden")
                    nc.vector.reciprocal_approx_fast(out=rden, in_=ps_den)
                    rden_b = bcastp.tile([128, 512], f32, tag="rden_b")
                    nc.gpsimd.partition_broadcast(rden_b, rden)
                    nc.vector.tensor_mul(
                        out=ctxT[h][:, s0 : s0 + 512], in0=ps_ctx, in1=rden_b
                    )
                    # stage + exchange this head's ctx slices immediately so
                    # the last head's A2A overlaps the other head's attention
                    nc.sync.dma_start(
                        out=ctx_send[sb, h].rearrange("d p r -> p d r"),
                        in_=ctxT[h][:, s0 : s0 + 512].rearrange(
                            "p (d r) -> p d r", d=NCORES
                        ),
                    )
                    if sim_mode:
                        nc.sync.dma_start(
                            out=ctx_recv[sb, h], in_=ctx_send[sb, h]
                        )
                    else:
                        nc.gpsimd.collective_compute(
                            "AllToAll",
                            mybir.AluOpType.bypass,
                            replica_groups=[list(range(NCORES))],
                            ins=[ctx_send[sb, h].rearrange("d p r -> (d p) r")],
                            outs=[ctx_recv[sb, h].rearrange("c p r -> (c p) r")],
                        )

                if sb == 2:
                    # pair-0: its four A2As completed long ago; loads placed
                    # here on the sync queue cannot stall anything upstream.
                    emit_pair_loads(0)
                    emit_pair_proj(0)

            emit_pair_loads(1)
            emit_pair_proj(1, head_major=True)

    nc.finalize()
    return nc


def get_nc(debug=False, sim_mode=False):
    key = ("nc", debug, sim_mode)
    if key not in _CACHE:
        _CACHE[key] = _build_nc(debug=debug, sim_mode=sim_mode)
    return _CACHE[key]


def make_in_maps(hidden_states, ln_weight, ln_bias, qkv_weight, qkv_bias,
                 proj_weight, proj_bias):
    import ml_dtypes

    f4 = np.float32
    bf = ml_dtypes.bfloat16
    x = np.asarray(hidden_states, f4)[:, 0, :]                      # [S, HID]
    xT = np.ascontiguousarray(x.T)                                  # [HID, S]
    g = np.asarray(ln_weight, f4)
    b = np.asarray(ln_bias, f4)
    W = np.asarray(qkv_weight, f4)
    W1 = W * g[None, :]
    b1 = np.asarray(qkv_bias, f4) + W @ b
    W3 = W1.reshape(3, NH, HD, HID)
    b3 = b1.reshape(3, NH, HD)
    pw = np.asarray(proj_weight, f4)
    wpf = np.ascontiguousarray(pw.T).astype(bf)                     # [HID, HID]
    pbias = np.asarray(proj_bias, f4).reshape(1, HID).astype(bf)
    maskneg = np.triu(np.full((128, 128), MASKVAL, f4), 1).astype(bf)
    ident = np.eye(128, dtype=bf)
    ones_col = np.ones((128, 1), f4)
    ones_bf = np.ones((128, 1), bf)
    ones_row_bf = np.ones((1, 128), bf)

    in_maps = []
    for c in range(NCORES):
        hs = slice(HPC * c, HPC * (c + 1))
        Wq = W3[0, hs].reshape(OV, HID)
        Wk = W3[1, hs].reshape(OV, HID)
        Wv = W3[2, hs].reshape(OV, HID)
        Wqk = np.concatenate([Wq, Wk], 0)                           # [512, HID]
        in_maps.append({
            "xT": xT,
            "wqkT": np.ascontiguousarray(Wqk.T),
            "wvT": np.ascontiguousarray(Wv.T),
            "wpf": wpf,
            "rsum_qk": Wqk.sum(1).reshape(1, OQK),
            "rsum_v": Wv.sum(1).reshape(1, OV),
            "bqk": np.concatenate(
                [b3[0, hs].reshape(OV), b3[1, hs].reshape(OV)]
            ).reshape(1, OQK),
            "bv": b3[2, hs].reshape(1, OV),
            "pbias": pbias,
            "maskneg": maskneg,
            "ident": ident,
            "ones_col": ones_col,
            "ones_bf": ones_bf,
            "ones_row_bf": ones_row_bf,
        })
    return in_maps


def assemble(outs):
    """outs: list of per-core [NSB*RS_OUT, HID] arrays -> full [S, 1, HID]."""
    full = np.empty((S, HID), np.float32)
    for c in range(NCORES):
        o = outs[c]
        for sb in range(NSB):
            full[sb * 512 + c * RS_OUT : sb * 512 + (c + 1) * RS_OUT, :] = o[
                sb * RS_OUT : (sb + 1) * RS_OUT, :
            ]
    return full.reshape(S, 1, HID)


class _Runner:
    """Cached PJRT runner: jit once, keep per-core weight shards device-
    resident across calls (re-uploaded only when weight bytes change)."""

    # inputs that depend only on the weights/constants (cacheable on device)
    WEIGHT_NAMES = frozenset({
        "wqkT", "wvT", "wpf", "rsum_qk", "rsum_v", "bqk", "bv", "pbias",
        "maskneg", "ident", "ones_col", "ones_bf", "ones_row_bf",
    })

    def __init__(self, nc):
        import jax
        import concourse.mybir as mybir
        from concourse import bass2jax
        from concourse.bass2jax import _bass_exec_p, partition_id_tensor
        from jax.sharding import Mesh, PartitionSpec
        from jax.experimental.shard_map import shard_map

        bass2jax.install_neuronx_cc_hook()
        self.nc = nc
        self.jax = jax
        partition_name = (
            nc.partition_id_tensor.name if nc.partition_id_tensor else None
        )
        in_names, out_names, out_avals = [], [], []
        for alloc in nc.m.functions[0].allocations:
            if not isinstance(alloc, mybir.MemoryLocationSet):
                continue
            name = alloc.memorylocations[0].name
            if alloc.kind == "ExternalInput":
                if name != partition_name:
                    in_names.append(name)
            elif alloc.kind == "ExternalOutput":
                shape = tuple(alloc.tensor_shape)
                out_names.append(name)
                out_avals.append(
                    jax.core.ShapedArray(shape, mybir.dt.np(alloc.dtype))
                )
        self.in_names, self.out_names, self.out_avals = in_names, out_names, out_avals
        all_in_names = list(in_names) + list(out_names)
        if partition_name is not None:
            all_in_names.append(partition_name)

        def _body(*args):
            operands = list(args)
            if partition_name is not None:
                operands.append(partition_id_tensor())
            return tuple(
                _bass_exec_p.bind(
                    *operands,
                    out_avals=tuple(out_avals),
                    in_names=tuple(all_in_names),
                    out_names=tuple(out_names),
                    lowering_input_output_aliases=(),
                    sim_require_finite=True,
                    sim_require_nnan=True,
                    nc=nc,
                )
            )

        devices = jax.devices()[:NCORES]
        mesh = Mesh(np.asarray(devices), ("core",))
        nin = len(in_names) + len(out_names)
        self._fn = jax.jit(
            shard_map(
                _body,
                mesh=mesh,
                in_specs=(PartitionSpec("core"),) * nin,
                out_specs=(PartitionSpec("core"),) * len(out_names),
                check_rep=False,
            ),
            keep_unused=True,
        )
        self._zeros = [
            np.zeros((NCORES * a.shape[0], *a.shape[1:]), a.dtype)
            for a in out_avals
        ]
        self._weight_cache = {}  # name -> (fingerprint, device_array)

    def __call__(self, in_maps):
        concat = {}
        for i, name in enumerate(self.in_names):
            arr = np.concatenate([np.asarray(m[name]) for m in in_maps], axis=0)
            if name in self.WEIGHT_NAMES:
                fp = hash(arr.tobytes())
                cached = self._weight_cache.get(name)
                if cached is not None and cached[0] == fp:
                    concat[name] = cached[1]
                else:
                    dev = self.jax.device_put(arr)
                    self._weight_cache[name] = (fp, dev)
                    concat[name] = dev
            else:
                concat[name] = arr
        out_arrs = self._fn(*[concat[n] for n in self.in_names], *self._zeros)
        outs = []
        for c in range(NCORES):
            outs.append({
                name: np.asarray(out_arrs[i]).reshape(
                    NCORES, *self.out_avals[i].shape
                )[c]
                for i, name in enumerate(self.out_names)
            })
        return outs


def get_runner():
    if "runner" not in _CACHE:
        _CACHE["runner"] = _Runner(get_nc())
    return _CACHE["runner"]


def kernel(hidden_states, ln_weight, ln_bias, qkv_weight, qkv_bias,
           proj_weight, proj_bias):
    in_maps = make_in_maps(hidden_states, ln_weight, ln_bias, qkv_weight,
                           qkv_bias, proj_weight, proj_bias)
    outs = get_runner()(in_maps)
    return assemble([o["out"] for o in outs])


# revision 42
# speedup vs baseline: 1.0831x; 1.0831x over previous
"""Tensor-parallel MultiHeadAttention (LN + fused QKV + causal SDPA + proj)
for 8 Trainium2 NeuronCores.

Sharding: 2 heads per core. LayerNorm gamma/beta folded into qkv weights on
host; LN (x-mu)*rstd applied via rank-1 PSUM corrections + evacuation scaling.
QKV/scores matmuls run in fp32r; softmax probs, V, and the context are bf16.

Collective strategy: the CC transport is element-rate-bound (~17 G elem/s
regardless of dtype or reduce-op), so instead of ReduceScattering dense
[512,2048] proj partials (1M elements per s-block), each core AllToAlls its
rank-compressed context slices (128K elements per s-block: its 256 hidden
dims x each destination's 64 output rows). Every core then computes the FULL
output projection for its own 256 output rows against resident bf16 proj
weights and writes f32 output directly - no reduction collective at all.
Host reassembles the full [S,1,HID] output.
"""

import sys

sys.path.insert(0, "/opt/trn_rl_repo")

import math

import numpy as np

S, HID, NH, HD = 2048, 2048, 16, 128
EPS = 1e-5
NCORES = 8
HPC = NH // NCORES        # heads per core: 2
OQK = 2 * HPC * HD        # q+k rows per core: 512
OV = HPC * HD             # v rows per core: 256
KO = HID // 128           # contraction chunks: 16
NSB = S // 512            # s-blocks: 4
NTB = S // 128            # t-blocks: 16
RS_OUT = 512 // NCORES    # output rows per core per s-block: 64
SCALE = 1.0 / math.sqrt(HD)
MASKVAL = -30000.0

_CACHE = {}


def _build_nc(debug=False, sim_mode=False):
    import concourse.mybir as mybir
    import concourse.tile as tile
    from concourse import bacc
    from contextlib import ExitStack

    f32 = mybir.dt.float32
    f32r = mybir.dt.float32r
    bf16 = mybir.dt.bfloat16
    Act = mybir.ActivationFunctionType

    nc = bacc.Bacc(num_devices=NCORES)

    # ---- I/O ----
    xT_d = nc.dram_tensor("xT", [HID, S], f32r, kind="ExternalInput")
    wqkT_d = nc.dram_tensor("wqkT", [HID, OQK], f32r, kind="ExternalInput")
    wvT_d = nc.dram_tensor("wvT", [HID, OV], f32r, kind="ExternalInput")
    wpf_d = nc.dram_tensor("wpf", [HID, HID], bf16, kind="ExternalInput")
    rsum_qk_d = nc.dram_tensor("rsum_qk", [1, OQK], f32r, kind="ExternalInput")
    rsum_v_d = nc.dram_tensor("rsum_v", [1, OV], f32r, kind="ExternalInput")
    bqk_d = nc.dram_tensor("bqk", [1, OQK], f32r, kind="ExternalInput")
    bv_d = nc.dram_tensor("bv", [1, OV], f32r, kind="ExternalInput")
    pbias_d = nc.dram_tensor("pbias", [1, HID], bf16, kind="ExternalInput")
    maskneg_d = nc.dram_tensor("maskneg", [128, 128], bf16, kind="ExternalInput")
    ident_d = nc.dram_tensor("ident", [128, 128], bf16, kind="ExternalInput")
    ones_d = nc.dram_tensor("ones_col", [128, 1], f32r, kind="ExternalInput")
    onesb_d = nc.dram_tensor("ones_bf", [128, 1], bf16, kind="ExternalInput")
    onesrow_d = nc.dram_tensor("ones_row_bf", [1, 128], bf16, kind="ExternalInput")
    out_d = nc.dram_tensor("out", [NSB * RS_OUT, HID], f32, kind="ExternalOutput")

    # internal DRAM: stats round trips + ctx exchange
    rstd_dram = nc.dram_tensor("rstd_scratch", [NSB, 512], f32)
    ctx_send = nc.dram_tensor(
        "ctx_send", [NSB, HPC, NCORES, 128, RS_OUT], bf16
    )
    ctx_recv = nc.dram_tensor(
        "ctx_recv", [NSB, HPC, NCORES, 128, RS_OUT], bf16
    )
    warm_s = nc.dram_tensor("warm_s", [NCORES, 64], bf16)
    warm_r = nc.dram_tensor("warm_r", [NCORES, 64], bf16)

    ctx = ExitStack()
    with ctx:
        tc = ctx.enter_context(tile.TileContext(nc))
        # resident pools (whole kernel lifetime)
        wpool = ctx.enter_context(tc.tile_pool(name="wpool", bufs=1))
        rows = ctx.enter_context(tc.tile_pool(name="rows", bufs=1))
        bigout = ctx.enter_context(tc.tile_pool(name="bigout", bufs=1))
        statrow = ctx.enter_context(tc.tile_pool(name="statrow", bufs=1))

        # ---- small resident rows ----
        ones_col = rows.tile([128, 1], f32r)
        nc.sync.dma_start(out=ones_col, in_=ones_d[:, :])
        ones_bf = rows.tile([128, 1], bf16)
        nc.sync.dma_start(out=ones_bf, in_=onesb_d[:, :])
        ones_row = rows.tile([1, 128], bf16)
        nc.sync.dma_start(out=ones_row, in_=onesrow_d[:, :])
        pbias_row = rows.tile([1, HID], bf16)
        nc.sync.dma_start(out=pbias_row, in_=pbias_d[:, :])
        rsum_qk = rows.tile([1, OQK], f32r)
        nc.sync.dma_start(out=rsum_qk, in_=rsum_qk_d[:, :])
        rsum_v = rows.tile([1, OV], f32r)
        nc.sync.dma_start(out=rsum_v, in_=rsum_v_d[:, :])
        bqk = rows.tile([1, OQK], f32r)
        nc.sync.dma_start(out=bqk, in_=bqk_d[:, :])
        bv = rows.tile([1, OV], f32r)
        nc.sync.dma_start(out=bv, in_=bv_d[:, :])
        eps_tile = rows.tile([128, 1], f32)
        nc.vector.memset(eps_tile, EPS)
        maskneg = rows.tile([128, 128], bf16)
        ident = rows.tile([128, 128], bf16)
        wpf = wpool.tile([128, KO, HID], bf16)

        # ---- persistent phase-1 outputs ----
        kT = [bigout.tile([128, S], f32r, name=f"kT{h}") for h in range(HPC)]
        vtile = bigout.tile([128, NTB, OV], bf16, name="vtile")
        ctxT = [bigout.tile([128, S], bf16, name=f"ctxT{h}") for h in range(HPC)]
        rstd_col = bigout.tile([128, NSB * 4], f32, name="rstd_col")
        ctxall = [
            bigout.tile([128, HPC, NCORES, 128], bf16, name=f"ctxall{p}")
            for p in range(2)
        ]

        # =========================================================
        # Per-sb pipeline: phase1(sb) -> attention(sb) -> ctx A2A(sb).
        # Final proj runs per sb-pair once that pair's A2A has landed.
        # =========================================================
        with (
            tc.tile_pool(name="wqkv", bufs=1) as wqkv,
            tc.tile_pool(name="xpool", bufs=2) as xpool,
            tc.tile_pool(name="qcur", bufs=2) as qpool,
            tc.tile_pool(name="sqpool", bufs=2) as sqpool,
            tc.tile_pool(name="rowr", bufs=1) as rowr,
            tc.tile_pool(name="bcast", bufs=1) as bcastp,
            tc.tile_pool(name="exppool", bufs=4) as exppool,
            tc.tile_pool(name="outpool", bufs=2) as outpool,
            tc.tile_pool(name="ps", bufs=8, space="PSUM") as psp,
        ):
            wqkT = wqkv.tile([128, KO, OQK], f32r)
            wvT = wqkv.tile([128, KO, OV], f32r)
            nc.sync.dma_start(out=maskneg, in_=maskneg_d[:, :])
            nc.sync.dma_start(out=ident, in_=ident_d[:, :])

            # warm-up collective: absorbs the runtime barrier, core-skew and
            # first-op ramp during the input-DMA window, so the first real
            # ctx exchange runs at steady-state cost.
            if not sim_mode:
                wtile = rows.tile([NCORES, 64], bf16, name="wtile")
                nc.vector.memset(wtile, 0.0)
                nc.sync.dma_start(out=warm_s[:, :], in_=wtile)
                nc.gpsimd.collective_compute(
                    "AllToAll",
                    mybir.AluOpType.bypass,
                    replica_groups=[list(range(NCORES))],
                    ins=[warm_s.ap()],
                    outs=[warm_r.ap()],
                )


            def emit_pair_loads(p, heads=(0, 1)):
                """SBUF loads of the gathered ctx^T for pair p, on the sync
                queue at a program point where the A2As have completed."""
                for j in range(2):
                    sbx = 2 * p + j
                    for h in heads:
                        nc.sync.dma_start(
                            out=ctxall[p][:, h, :, j * 64 : (j + 1) * 64],
                            in_=ctx_recv[sbx, h].rearrange("c p r -> p c r"),
                        )

            def emit_pair_proj(p, head_major=False):
                """Full-depth output projection for rows of s-blocks
                (2p, 2p+1): lhsT = gathered ctx^T [128 hid-chunk, 128 rows],
                rhs = resident bf16 proj weights; bias via rank-1 matmul.
                head_major runs all heads' h0 chunks first across 4 PSUM
                banks so the tail pair only waits on the last head's A2A."""
                if head_major:
                    ps_os = [
                        psp.tile([128, 512], f32, tag="bank", name=f"ps_o{p}_{ob}")
                        for ob in range(4)
                    ]
                    for h in range(HPC):
                        for ob in range(4):
                            o0 = ob * 512
                            for c in range(NCORES):
                                nc.tensor.matmul(
                                    ps_os[ob],
                                    ctxall[p][:, h, c, :],
                                    wpf[:, c * HPC + h, o0 : o0 + 512],
                                    start=(h == 0 and c == 0),
                                    stop=False,
                                )
                    for ob in range(4):
                        o0 = ob * 512
                        nc.tensor.matmul(
                            ps_os[ob],
                            ones_row,
                            pbias_row[0:1, o0 : o0 + 512],
                            start=False,
                            stop=True,
                        )
                        otile = outpool.tile([128, 512], f32, tag="otile")
                        nc.vector.tensor_copy(out=otile, in_=ps_os[ob])
                        nc.sync.dma_start(
                            out=out_d[p * 128 : (p + 1) * 128, o0 : o0 + 512],
                            in_=otile,
                        )
                else:
                    for ob in range(4):
                        o0 = ob * 512
                        ps_o = psp.tile(
                            [128, 512], f32, tag="bank", name=f"ps_o{p}_{ob}"
                        )
                        for h in range(HPC):
                            for c in range(NCORES):
                                nc.tensor.matmul(
                                    ps_o,
                                    ctxall[p][:, h, c, :],
                                    wpf[:, c * HPC + h, o0 : o0 + 512],
                                    start=(h == 0 and c == 0),
                                    stop=False,
                                )
                        nc.tensor.matmul(
                            ps_o,
                            ones_row,
                            pbias_row[0:1, o0 : o0 + 512],
                            start=False,
                            stop=True,
                        )
                        otile = outpool.tile([128, 512], f32, tag="otile")
                        nc.vector.tensor_copy(out=otile, in_=ps_o)
                        nc.sync.dma_start(
                            out=out_d[p * 128 : (p + 1) * 128, o0 : o0 + 512],
                            in_=otile,
                        )

            for sb in range(NSB):
                s0 = sb * 512
                # ---------------- phase 1: stats + q/k/v ----------------
                ps_sums = psp.tile([1, 512], f32, tag="bank", name="ps_sums")
                ps_sumsq = psp.tile([1, 512], f32, tag="bank", name="ps_sumsq")
                ps_qk = [
                    psp.tile([128, 512], f32, tag="bank", name=f"ps_qk{ob}")
                    for ob in range(4)
                ]
                # two banks, each packing two 256-wide v accumulation groups
                ps_v = [
                    psp.tile([128, 512], f32, tag="bank", name=f"ps_v{i}")
                    for i in range(2)
                ]
                for hq in range(8):
                    xt2 = xpool.tile([128, 2, 512], f32r, tag="xt",
                                     name=f"xt{sb}_{hq}")
                    nc.sync.dma_start(
                        out=xt2,
                        in_=xT_d[hq * 256 : (hq + 1) * 256, s0 : s0 + 512].rearrange(
                            "(c p) s -> p c s", p=128
                        ),
                    )
                    if sb == 0:
                        nc.sync.dma_start(
                            out=wqkT[:, hq * 2 : (hq + 1) * 2, :],
                            in_=wqkT_d[hq * 256 : (hq + 1) * 256, :].rearrange(
                                "(c p) o -> p c o", p=128
                            ),
                        )
                        if hq % 2 == 0:
                            nc.sync.dma_start(
                                out=wvT[:, hq * 2 : (hq + 2) * 2, :],
                                in_=wvT_d[hq * 256 : (hq + 2) * 256, :].rearrange(
                                    "(c p) o -> p c o", p=128
                                ),
                            )
                    for hh in range(2):
                        h = hq * 2 + hh
                        xt = xt2[:, hh, :]
                        xsq = sqpool.tile([128, 512], f32r, tag="xsq")
                        if h % 2 == 0:
                            nc.scalar.activation(out=xsq, in_=xt, func=Act.Square)
                        else:
                            nc.vector.tensor_mul(out=xsq, in0=xt, in1=xt)
                        nc.tensor.matmul(
                            ps_sums, ones_col, xt, start=(h == 0), stop=(h == KO - 1)
                        )
                        nc.tensor.matmul(
                            ps_sumsq, ones_col, xsq, start=(h == 0),
                            stop=(h == KO - 1)
                        )
                        for ob in range(4):
                            nc.tensor.matmul(
                                ps_qk[ob],
                                wqkT[:, h, ob * 128 : (ob + 1) * 128],
                                xt,
                                start=(h == 0),
                                stop=False,
                            )
                        for vs in range(4):
                            nc.tensor.matmul(
                                ps_v[vs // 2][:, (vs % 2) * 256 : (vs % 2 + 1) * 256],
                                xt[:, vs * 128 : (vs + 1) * 128],
                                wvT[:, h, :],
                                start=(h == 0 and vs % 2 == 0),
                                stop=False,
                                skip_group_check=(vs % 2 == 1),
                            )
                if sb < 2:
                    # proj weights in 2.1MB chunks during the attention
                    # phases of sb0/sb1 (quiet DMA windows; needed ~250us)
                    for k in range(2):
                        cc0 = (sb * 2 + k) * 4
                        nc.sync.dma_start(
                            out=wpf[:, cc0 : cc0 + 4, :],
                            in_=wpf_d[cc0 * 128 : (cc0 + 4) * 128, :].rearrange(
                                "(c p) o -> p c o", p=128
                            ),
                        )

                # stats rows (short critical chain)
                negmu_r = rowr.tile([1, 512], f32r, tag="negmu_r")
                nc.vector.tensor_scalar_mul(
                    out=negmu_r, in0=ps_sums, scalar1=-1.0 / HID
                )
                mu = statrow.tile([1, 512], f32, tag="mu")
                nc.vector.tensor_scalar_mul(out=mu, in0=ps_sums, scalar1=1.0 / HID)
                nc.vector.tensor_mul(out=mu, in0=mu, in1=mu)  # mu := mu^2
                var = statrow.tile([1, 512], f32, tag="var")
                nc.vector.scalar_tensor_tensor(
                    out=var,
                    in0=ps_sumsq,
                    scalar=1.0 / HID,
                    in1=mu,
                    op0=mybir.AluOpType.mult,
                    op1=mybir.AluOpType.subtract,
                )
                invrstd_r = rowr.tile([1, 512], f32r, tag="invrstd_r")
                nc.scalar.activation(
                    out=invrstd_r, in_=var, func=Act.Sqrt, bias=eps_tile[0:1]
                )
                rstd = statrow.tile([1, 512], f32, tag="rstd")
                nc.vector.reciprocal_approx_fast(
                    out=rstd, in_=invrstd_r.bitcast(f32)
                )

                # rstd column layout (DRAM bounce) + partition broadcast
                nc.sync.dma_start(out=rstd_dram[sb : sb + 1, :], in_=rstd)
                nc.sync.dma_start(
                    out=rstd_col[:, sb * 4 : (sb + 1) * 4],
                    in_=rstd_dram[sb, :].rearrange("(f p) -> p f", p=128),
                )
                rstd_b = bcastp.tile([128, 512], f32, tag="rstd_b")
                nc.gpsimd.partition_broadcast(rstd_b, rstd)

                # q/k rank-1 corrections + evac (q transient, k persistent)
                qcur = [
                    qpool.tile([128, 512], f32r, tag=f"q{h}", name=f"q{sb}_{h}")
                    for h in range(HPC)
                ]
                for ob in range(4):
                    nc.tensor.matmul(
                        ps_qk[ob],
                        rsum_qk[0:1, ob * 128 : (ob + 1) * 128],
                        negmu_r,
                        start=False,
                        stop=False,
                    )
                    nc.tensor.matmul(
                        ps_qk[ob],
                        bqk[0:1, ob * 128 : (ob + 1) * 128],
                        invrstd_r,
                        start=False,
                        stop=True,
                    )
                    if ob < 2:
                        nc.vector.tensor_mul(
                            out=qcur[ob], in0=ps_qk[ob], in1=rstd_b
                        )
                    else:
                        nc.vector.tensor_mul(
                            out=kT[ob - 2][:, s0 : s0 + 512],
                            in0=ps_qk[ob],
                            in1=rstd_b,
                        )

                # v rank-1 corrections + evac (bf16 out)
                for vs in range(4):
                    pv = ps_v[vs // 2][:, (vs % 2) * 256 : (vs % 2 + 1) * 256]
                    nc.tensor.matmul(
                        pv,
                        negmu_r[0:1, vs * 128 : (vs + 1) * 128],
                        rsum_v,
                        start=False,
                        stop=False,
                        skip_group_check=True,
                    )
                    nc.tensor.matmul(
                        pv,
                        invrstd_r[0:1, vs * 128 : (vs + 1) * 128],
                        bv,
                        start=False,
                        stop=True,
                        skip_group_check=True,
                    )
                    nc.vector.tensor_scalar_mul(
                        out=vtile[:, sb * 4 + vs, :],
                        in0=pv,
                        scalar1=rstd_col[:, sb * 4 + vs : sb * 4 + vs + 1],
                    )

                # ---------------- attention for this sb ----------------
                ntb = 4 * (sb + 1)  # causal t-blocks
                for h in range(HPC):
                    ps_ctx = psp.tile(
                        [128, 512], f32, tag="bank", name=f"ps_ctx{sb}_{h}"
                    )
                    ps_den = psp.tile(
                        [1, 512], f32, tag="bank", name=f"ps_den{sb}_{h}"
                    )
                    for tb in range(ntb):
                        t0 = tb * 128
                        delta = max(0, t0 - s0)
                        ps_sc = psp.tile([128, 512], f32, tag="bank", name="ps_sc")
                        nc.tensor.matmul(
                            ps_sc[:, delta:512],
                            kT[h][:, t0 : t0 + 128],
                            qcur[h][:, delta:512],
                            start=True,
                            stop=(t0 < s0),
                        )
                        if t0 >= s0:
                            nc.tensor.matmul(
                                ps_sc[:, delta : delta + 128],
                                maskneg,
                                ident,
                                start=False,
                                stop=True,
                            )
                        expt = exppool.tile([128, 512], bf16, tag="expt")
                        nc.scalar.activation(
                            out=expt[:, delta:512],
                            in_=ps_sc[:, delta:512],
                            func=Act.Exp,
                            scale=SCALE,
                        )
                        # columns [0, delta) are invalid (t > s) and never
                        # written: every column's first accumulant is tb==0.
                        nc.tensor.matmul(
                            ps_ctx[:, delta:512],
                            vtile[:, tb, h * HD : (h + 1) * HD],
                            expt[:, delta:512],
                            start=(tb == 0),
                            stop=(tb == ntb - 1),
                            skip_group_check=True,
                        )
                        nc.tensor.matmul(
                            ps_den[:, delta:512],
                            ones_bf,
                            expt[:, delta:512],
                            start=(tb == 0),
                            stop=(tb == ntb - 1),
                            skip_group_check=True,
                        )
                    rden = statrow.tile([1, 512], f32, tag="rden")
                    nc.vector.reciprocal_approx_fast(out=rden, in_=ps_den)
                    rden_b = bcastp.tile([128, 512], f32, tag="rden_b")
                    nc.gpsimd.partition_broadcast(rden_b, rden)
                    nc.vector.tensor_mul(
                        out=ctxT[h][:, s0 : s0 + 512], in0=ps_ctx, in1=rden_b
                    )
                    # stage + exchange this head's ctx slices immediately so
                    # the last head's A2A overlaps the other head's attention
                    nc.sync.dma_start(
                        out=ctx_send[sb, h].rearrange("d p r -> p d r"),
                        in_=ctxT[h][:, s0 : s0 + 512].rearrange(
                            "p (d r) -> p d r", d=NCORES
                        ),
                    )
                    if sim_mode:
                        nc.sync.dma_start(
                            out=ctx_recv[sb, h], in_=ctx_send[sb, h]
                        )
                    else:
                        nc.gpsimd.collective_compute(
                            "AllToAll",
                            mybir.AluOpType.bypass,
                            replica_groups=[list(range(NCORES))],
                            ins=[ctx_send[sb, h].rearrange("d p r -> (d p) r")],
                            outs=[ctx_recv[sb, h].rearrange("c p r -> (c p) r")],
                        )

                if sb == 2:
                    # pair-0: its four A2As completed long ago; loads placed
                    # here on the sync queue cannot stall anything upstream.
                    emit_pair_loads(0)
                    emit_pair_proj(0)

            emit_pair_loads(1)
            emit_pair_proj(1, head_major=True)

    nc.finalize()
    return nc


def get_nc(debug=False, sim_mode=False):
    key = ("nc", debug, sim_mode)
    if key not in _CACHE:
        _CACHE[key] = _build_nc(debug=debug, sim_mode=sim_mode)
    return _CACHE[key]


def make_in_maps(hidden_states, ln_weight, ln_bias, qkv_weight, qkv_bias,
                 proj_weight, proj_bias):
    import ml_dtypes

    f4 = np.float32
    bf = ml_dtypes.bfloat16
    x = np.asarray(hidden_states, f4)[:, 0, :]                      # [S, HID]
    xT = np.ascontiguousarray(x.T)                                  # [HID, S]
    g = np.asarray(ln_weight, f4)
    b = np.asarray(ln_bias, f4)
    W = np.asarray(qkv_weight, f4)
    W1 = W * g[None, :]
    b1 = np.asarray(qkv_bias, f4) + W @ b
    W3 = W1.reshape(3, NH, HD, HID)
    b3 = b1.reshape(3, NH, HD)
    pw = np.asarray(proj_weight, f4)
    wpf = np.ascontiguousarray(pw.T).astype(bf)                     # [HID, HID]
    pbias = np.asarray(proj_bias, f4).reshape(1, HID).astype(bf)
    maskneg = np.triu(np.full((128, 128), MASKVAL, f4), 1).astype(bf)
    ident = np.eye(128, dtype=bf)
    ones_col = np.ones((128, 1), f4)
    ones_bf = np.ones((128, 1), bf)
    ones_row_bf = np.ones((1, 128), bf)

    in_maps = []
    for c in range(NCORES):
        hs = slice(HPC * c, HPC * (c + 1))
        Wq = W3[0, hs].reshape(OV, HID)
        Wk = W3[1, hs].reshape(OV, HID)
        Wv = W3[2, hs].reshape(OV, HID)
        Wqk = np.concatenate([Wq, Wk], 0)                           # [512, HID]
        in_maps.append({
            "xT": xT,
            "wqkT": np.ascontiguousarray(Wqk.T),
            "wvT": np.ascontiguousarray(Wv.T),
            "wpf": wpf,
            "rsum_qk": Wqk.sum(1).reshape(1, OQK),
            "rsum_v": Wv.sum(1).reshape(1, OV),
            "bqk": np.concatenate(
                [b3[0, hs].reshape(OV), b3[1, hs].reshape(OV)]
            ).reshape(1, OQK),
            "bv": b3[2, hs].reshape(1, OV),
            "pbias": pbias,
            "maskneg": maskneg,
            "ident": ident,
            "ones_col": ones_col,
            "ones_bf": ones_bf,
            "ones_row_bf": ones_row_bf,
        })
    return in_maps


def assemble(outs):
    """outs: list of per-core [NSB*RS_OUT, HID] arrays -> full [S, 1, HID]."""
    full = np.empty((S, HID), np.float32)
    for c in range(NCORES):
        o = outs[c]
        for sb in range(NSB):
            full[sb * 512 + c * RS_OUT : sb * 512 + (c + 1) * RS_OUT, :] = o[
                sb * RS_OUT : (sb + 1) * RS_OUT, :
            ]
    return full.reshape(S, 1, HID)


class _Runner:
    """Cached PJRT runner: jit once, keep per-core weight shards device-
    resident across calls (re-uploaded only when weight bytes change)."""

    # inputs that depend only on the weights/constants (cacheable on device)
    WEIGHT_NAMES = frozenset({
        "wqkT", "wvT", "wpf", "rsum_qk", "rsum_v", "bqk", "bv", "pbias",
        "maskneg", "ident", "ones_col", "ones_bf", "ones_row_bf",
    })

    def __init__(self, nc):
        import jax
        import concourse.mybir as mybir
        from concourse import bass2jax
        from concourse.bass2jax import _bass_exec_p, partition_id_tensor
        from jax.sharding import Mesh, PartitionSpec
        from jax.experimental.shard_map import shard_map

        bass2jax.install_neuronx_cc_hook()
        self.nc = nc
        self.jax = jax
        partition_name = (
            nc.partition_id_tensor.name if nc.partition_id_tensor else None
        )
        in_names, out_names, out_avals = [], [], []
        for alloc in nc.m.functions[0].allocations:
            if not isinstance(alloc, mybir.MemoryLocationSet):
                continue
            name = alloc.memorylocations[0].name
            if alloc.kind == "ExternalInput":
                if name != partition_name:
                    in_names.append(name)
            elif alloc.kind == "ExternalOutput":
                shape = tuple(alloc.tensor_shape)
                out_names.append(name)
                out_avals.append(
                    jax.core.ShapedArray(shape, mybir.dt.np(alloc.dtype))
                )
        self.in_names, self.out_names, self.out_avals = in_names, out_names, out_avals
        all_in_names = list(in_names) + list(out_names)
        if partition_name is not None:
            all_in_names.append(partition_name)

        def _body(*args):
            operands = list(args)
            if partition_name is not None:
                operands.append(partition_id_tensor())
            return tuple(
                _bass_exec_p.bind(
                    *operands,
                    out_avals=tuple(out_avals),
                    in_names=tuple(all_in_names),
                    out_names=tuple(out_names),
                    lowering_input_output_aliases=(),
                    sim_require_finite=True,
                    sim_require_nnan=True,
                    nc=nc,
                )
            )

        devices = jax.devices()[:NCORES]
        mesh = Mesh(np.asarray(devices), ("core",))
        nin = len(in_names) + len(out_names)
        self._fn = jax.jit(
            shard_map(
                _body,
                mesh=mesh,
                in_specs=(PartitionSpec("core"),) * nin,
                out_specs=(PartitionSpec("core"),) * len(out_names),
                check_rep=False,
            ),
            keep_unused=True,
        )
        self._zeros = [
            np.zeros((NCORES * a.shape[0], *a.shape[1:]), a.dtype)
            for a in out_avals
        ]
        self._weight_cache = {}  # name -> (fingerprint, device_array)
        self._warmed = False

    def __call__(self, in_maps):
        concat = {}
        for i, name in enumerate(self.in_names):
            arr = np.concatenate([np.asarray(m[name]) for m in in_maps], axis=0)
            if name in self.WEIGHT_NAMES:
                fp = hash(arr.tobytes())
                cached = self._weight_cache.get(name)
                if cached is not None and cached[0] == fp:
                    concat[name] = cached[1]
                else:
                    dev = self.jax.device_put(arr)
                    self._weight_cache[name] = (fp, dev)
                    concat[name] = dev
            else:
                concat[name] = arr
        args = [concat[n] for n in self.in_names]
        if not self._warmed:
            # first-ever NEFF execution can race device/semaphore init;
            # run once to warm, take results from the steady-state run
            self.jax.block_until_ready(self._fn(*args, *self._zeros))
            self._warmed = True
        out_arrs = self._fn(*args, *self._zeros)
        outs = []
        for c in range(NCORES):
            outs.append({
                name: np.asarray(out_arrs[i]).reshape(
                    NCORES, *self.out_avals[i].shape
                )[c]
                for i, name in enumerate(self.out_names)
            })
        return outs


def get_runner():
    if "runner" not in _CACHE:
        _CACHE["runner"] = _Runner(get_nc())
    return _CACHE["runner"]


def kernel(hidden_states, ln_weight, ln_bias, qkv_weight, qkv_bias,
           proj_weight, proj_bias):
    in_maps = make_in_maps(hidden_states, ln_weight, ln_bias, qkv_weight,
                           qkv_bias, proj_weight, proj_bias)
    outs = get_runner()(in_maps)
    return assemble([o["out"] for o in outs])


# revision 46
# speedup vs baseline: 1.1072x; 1.0223x over previous
"""Tensor-parallel MultiHeadAttention (LN + fused QKV + causal SDPA + proj)
for 8 Trainium2 NeuronCores.

Sharding: 2 heads per core. LayerNorm gamma/beta folded into qkv weights on
host; LN (x-mu)*rstd applied via rank-1 PSUM corrections + evacuation scaling.
QKV/scores matmuls run in fp32r; softmax probs, V, and the context are bf16.

Collective strategy: the CC transport is element-rate-bound (~17 G elem/s
regardless of dtype or reduce-op), so instead of ReduceScattering dense
[512,2048] proj partials (1M elements per s-block), each core AllToAlls its
rank-compressed context slices (128K elements per s-block: its 256 hidden
dims x each destination's 64 output rows). Every core then computes the FULL
output projection for its own 256 output rows against resident bf16 proj
weights and writes f32 output directly - no reduction collective at all.
Host reassembles the full [S,1,HID] output.
"""

import sys

sys.path.insert(0, "/opt/trn_rl_repo")

import math

import numpy as np

S, HID, NH, HD = 2048, 2048, 16, 128
EPS = 1e-5
NCORES = 8
HPC = NH // NCORES        # heads per core: 2
OQK = 2 * HPC * HD        # q+k rows per core: 512
OV = HPC * HD             # v rows per core: 256
KO = HID // 128           # contraction chunks: 16
NSB = S // 512            # s-blocks: 4
NTB = S // 128            # t-blocks: 16
RS_OUT = 512 // NCORES    # output rows per core per s-block: 64
SCALE = 1.0 / math.sqrt(HD)
MASKVAL = -30000.0

_CACHE = {}


def _build_nc(debug=False, sim_mode=False):
    import concourse.mybir as mybir
    import concourse.tile as tile
    from concourse import bacc
    from contextlib import ExitStack

    f32 = mybir.dt.float32
    f32r = mybir.dt.float32r
    bf16 = mybir.dt.bfloat16
    Act = mybir.ActivationFunctionType

    nc = bacc.Bacc(num_devices=NCORES)

    # ---- I/O ----
    xT_d = nc.dram_tensor("xT", [HID, S], f32r, kind="ExternalInput")
    wqkT_d = nc.dram_tensor("wqkT", [HID, OQK], f32r, kind="ExternalInput")
    wvT_d = nc.dram_tensor("wvT", [HID, OV], f32r, kind="ExternalInput")
    wpf_d = nc.dram_tensor("wpf", [HID, HID], bf16, kind="ExternalInput")
    rsum_qk_d = nc.dram_tensor("rsum_qk", [1, OQK], f32r, kind="ExternalInput")
    rsum_v_d = nc.dram_tensor("rsum_v", [1, OV], f32r, kind="ExternalInput")
    bqk_d = nc.dram_tensor("bqk", [1, OQK], f32r, kind="ExternalInput")
    bv_d = nc.dram_tensor("bv", [1, OV], f32r, kind="ExternalInput")
    pbias_d = nc.dram_tensor("pbias", [1, HID], bf16, kind="ExternalInput")
    maskneg_d = nc.dram_tensor("maskneg", [128, 128], bf16, kind="ExternalInput")
    ident_d = nc.dram_tensor("ident", [128, 128], bf16, kind="ExternalInput")
    ones_d = nc.dram_tensor("ones_col", [128, 1], f32r, kind="ExternalInput")
    onesb_d = nc.dram_tensor("ones_bf", [128, 1], bf16, kind="ExternalInput")
    onesrow_d = nc.dram_tensor("ones_row_bf", [1, 128], bf16, kind="ExternalInput")
    out_d = nc.dram_tensor("out", [NSB * RS_OUT, HID], f32, kind="ExternalOutput")

    # internal DRAM: stats round trips + ctx exchange
    rstd_dram = nc.dram_tensor("rstd_scratch", [NSB, 512], f32)
    ctx_send = nc.dram_tensor(
        "ctx_send", [NSB, HPC, NCORES, 128, RS_OUT], bf16
    )
    ctx_recv = nc.dram_tensor(
        "ctx_recv", [NSB, HPC, NCORES, 128, RS_OUT], bf16
    )
    warm_s = nc.dram_tensor("warm_s", [NCORES, 64], bf16)
    warm_r = nc.dram_tensor("warm_r", [NCORES, 64], bf16)

    ctx = ExitStack()
    with ctx:
        tc = ctx.enter_context(tile.TileContext(nc))
        # resident pools (whole kernel lifetime)
        wpool = ctx.enter_context(tc.tile_pool(name="wpool", bufs=1))
        rows = ctx.enter_context(tc.tile_pool(name="rows", bufs=1))
        bigout = ctx.enter_context(tc.tile_pool(name="bigout", bufs=1))
        statrow = ctx.enter_context(tc.tile_pool(name="statrow", bufs=1))

        # ---- small resident rows ----
        ones_col = rows.tile([128, 1], f32r)
        nc.sync.dma_start(out=ones_col, in_=ones_d[:, :])
        ones_bf = rows.tile([128, 1], bf16)
        nc.sync.dma_start(out=ones_bf, in_=onesb_d[:, :])
        ones_row = rows.tile([1, 128], bf16)
        nc.sync.dma_start(out=ones_row, in_=onesrow_d[:, :])
        pbias_row = rows.tile([1, HID], bf16)
        nc.sync.dma_start(out=pbias_row, in_=pbias_d[:, :])
        rsum_qk = rows.tile([1, OQK], f32r)
        nc.sync.dma_start(out=rsum_qk, in_=rsum_qk_d[:, :])
        rsum_v = rows.tile([1, OV], f32r)
        nc.sync.dma_start(out=rsum_v, in_=rsum_v_d[:, :])
        bqk = rows.tile([1, OQK], f32r)
        nc.sync.dma_start(out=bqk, in_=bqk_d[:, :])
        bv = rows.tile([1, OV], f32r)
        nc.sync.dma_start(out=bv, in_=bv_d[:, :])
        eps_tile = rows.tile([128, 1], f32)
        nc.vector.memset(eps_tile, EPS)
        maskneg = rows.tile([128, 128], bf16)
        ident = rows.tile([128, 128], bf16)
        wpf = wpool.tile([128, KO, HID], bf16)

        # ---- persistent phase-1 outputs ----
        kT = [bigout.tile([128, S], f32r, name=f"kT{h}") for h in range(HPC)]
        vtile = bigout.tile([128, NTB, OV], bf16, name="vtile")
        ctxT = [bigout.tile([128, S], bf16, name=f"ctxT{h}") for h in range(HPC)]
        rstd_col = bigout.tile([128, NSB * 4], f32, name="rstd_col")


        # =========================================================
        # Per-sb pipeline: phase1(sb) -> attention(sb) -> ctx A2A(sb).
        # Final proj runs per sb-pair once that pair's A2A has landed.
        # =========================================================
        with (
            tc.tile_pool(name="wqkv", bufs=1) as wqkv,
            tc.tile_pool(name="xpool", bufs=3) as xpool,
            tc.tile_pool(name="capool", bufs=1) as capool,
            tc.tile_pool(name="qcur", bufs=2) as qpool,
            tc.tile_pool(name="sqpool", bufs=2) as sqpool,
            tc.tile_pool(name="rowr", bufs=1) as rowr,
            tc.tile_pool(name="bcast", bufs=1) as bcastp,
            tc.tile_pool(name="exppool", bufs=4) as exppool,
            tc.tile_pool(name="outpool", bufs=2) as outpool,
            tc.tile_pool(name="ps", bufs=8, space="PSUM") as psp,
        ):
            wqkT = wqkv.tile([128, KO, OQK], f32r)
            wvT = wqkv.tile([128, KO, OV], f32r)
            nc.sync.dma_start(out=maskneg, in_=maskneg_d[:, :])
            nc.sync.dma_start(out=ident, in_=ident_d[:, :])

            # warm-up collective: absorbs the runtime barrier, core-skew and
            # first-op ramp during the input-DMA window, so the first real
            # ctx exchange runs at steady-state cost.
            if not sim_mode:
                wtile = rows.tile([NCORES, 64], bf16, name="wtile")
                nc.vector.memset(wtile, 0.0)
                nc.sync.dma_start(out=warm_s[:, :], in_=wtile)
                nc.gpsimd.collective_compute(
                    "AllToAll",
                    mybir.AluOpType.bypass,
                    replica_groups=[list(range(NCORES))],
                    ins=[warm_s.ap()],
                    outs=[warm_r.ap()],
                )


            def emit_pair_loads(p):
                """SBUF loads of the gathered ctx^T for pair p. On the gpsimd
                queue: even when the scheduler hoists them to right after
                their A2A triggers, nothing that matters queues behind them
                there. One shared buffer (pairs are time-disjoint)."""
                ca = capool.tile(
                    [128, HPC, NCORES, 128], bf16, tag="ca", name=f"ctxall{p}"
                )
                for j in range(2):
                    sbx = 2 * p + j
                    for h in range(HPC):
                        nc.gpsimd.dma_start(
                            out=ca[:, h, :, j * 64 : (j + 1) * 64],
                            in_=ctx_recv[sbx, h].rearrange("c p r -> p c r"),
                        )
                return ca

            def emit_pair_proj(p, ca, head_major=False):
                """Full-depth output projection for rows of s-blocks
                (2p, 2p+1): lhsT = gathered ctx^T [128 hid-chunk, 128 rows],
                rhs = resident bf16 proj weights; bias via rank-1 matmul.
                head_major runs all heads' h0 chunks first across 4 PSUM
                banks so the tail pair only waits on the last head's A2A."""
                if head_major:
                    ps_os = [
                        psp.tile([128, 512], f32, tag="bank", name=f"ps_o{p}_{ob}")
                        for ob in range(4)
                    ]
                    for h in range(HPC):
                        for ob in range(4):
                            o0 = ob * 512
                            for c in range(NCORES):
                                nc.tensor.matmul(
                                    ps_os[ob],
                                    ca[:, h, c, :],
                                    wpf[:, c * HPC + h, o0 : o0 + 512],
                                    start=(h == 0 and c == 0),
                                    stop=False,
                                )
                    for ob in range(4):
                        o0 = ob * 512
                        nc.tensor.matmul(
                            ps_os[ob],
                            ones_row,
                            pbias_row[0:1, o0 : o0 + 512],
                            start=False,
                            stop=True,
                        )
                        otile = outpool.tile([128, 512], f32, tag="otile")
                        nc.vector.tensor_copy(out=otile, in_=ps_os[ob])
                        nc.sync.dma_start(
                            out=out_d[p * 128 : (p + 1) * 128, o0 : o0 + 512],
                            in_=otile,
                        )
                else:
                    for ob in range(4):
                        o0 = ob * 512
                        ps_o = psp.tile(
                            [128, 512], f32, tag="bank", name=f"ps_o{p}_{ob}"
                        )
                        for h in range(HPC):
                            for c in range(NCORES):
                                nc.tensor.matmul(
                                    ps_o,
                                    ca[:, h, c, :],
                                    wpf[:, c * HPC + h, o0 : o0 + 512],
                                    start=(h == 0 and c == 0),
                                    stop=False,
                                )
                        nc.tensor.matmul(
                            ps_o,
                            ones_row,
                            pbias_row[0:1, o0 : o0 + 512],
                            start=False,
                            stop=True,
                        )
                        otile = outpool.tile([128, 512], f32, tag="otile")
                        nc.vector.tensor_copy(out=otile, in_=ps_o)
                        nc.sync.dma_start(
                            out=out_d[p * 128 : (p + 1) * 128, o0 : o0 + 512],
                            in_=otile,
                        )

            for sb in range(NSB):
                s0 = sb * 512
                # ---------------- phase 1: stats + q/k/v ----------------
                ps_sums = psp.tile([1, 512], f32, tag="bank", name="ps_sums")
                ps_sumsq = psp.tile([1, 512], f32, tag="bank", name="ps_sumsq")
                ps_qk = [
                    psp.tile([128, 512], f32, tag="bank", name=f"ps_qk{ob}")
                    for ob in range(4)
                ]
                # two banks, each packing two 256-wide v accumulation groups
                ps_v = [
                    psp.tile([128, 512], f32, tag="bank", name=f"ps_v{i}")
                    for i in range(2)
                ]
                for hq in range(8):
                    xt2 = xpool.tile([128, 2, 512], f32r, tag="xt",
                                     name=f"xt{sb}_{hq}")
                    nc.sync.dma_start(
                        out=xt2,
                        in_=xT_d[hq * 256 : (hq + 1) * 256, s0 : s0 + 512].rearrange(
                            "(c p) s -> p c s", p=128
                        ),
                    )
                    if sb == 0:
                        nc.sync.dma_start(
                            out=wqkT[:, hq * 2 : (hq + 1) * 2, :],
                            in_=wqkT_d[hq * 256 : (hq + 1) * 256, :].rearrange(
                                "(c p) o -> p c o", p=128
                            ),
                        )
                        if hq % 2 == 0:
                            nc.sync.dma_start(
                                out=wvT[:, hq * 2 : (hq + 2) * 2, :],
                                in_=wvT_d[hq * 256 : (hq + 2) * 256, :].rearrange(
                                    "(c p) o -> p c o", p=128
                                ),
                            )
                    for hh in range(2):
                        h = hq * 2 + hh
                        xt = xt2[:, hh, :]
                        xsq = sqpool.tile([128, 512], f32r, tag="xsq")
                        if h % 2 == 0:
                            nc.scalar.activation(out=xsq, in_=xt, func=Act.Square)
                        else:
                            nc.vector.tensor_mul(out=xsq, in0=xt, in1=xt)
                        nc.tensor.matmul(
                            ps_sums, ones_col, xt, start=(h == 0), stop=(h == KO - 1)
                        )
                        nc.tensor.matmul(
                            ps_sumsq, ones_col, xsq, start=(h == 0),
                            stop=(h == KO - 1)
                        )
                        for ob in range(4):
                            nc.tensor.matmul(
                                ps_qk[ob],
                                wqkT[:, h, ob * 128 : (ob + 1) * 128],
                                xt,
                                start=(h == 0),
                                stop=False,
                            )
                        for vs in range(4):
                            nc.tensor.matmul(
                                ps_v[vs // 2][:, (vs % 2) * 256 : (vs % 2 + 1) * 256],
                                xt[:, vs * 128 : (vs + 1) * 128],
                                wvT[:, h, :],
                                start=(h == 0 and vs % 2 == 0),
                                stop=False,
                                skip_group_check=(vs % 2 == 1),
                            )
                if sb < 2:
                    # proj weights in 2.1MB chunks during the attention
                    # phases of sb0/sb1 (quiet DMA windows; needed ~250us)
                    for k in range(2):
                        cc0 = (sb * 2 + k) * 4
                        nc.sync.dma_start(
                            out=wpf[:, cc0 : cc0 + 4, :],
                            in_=wpf_d[cc0 * 128 : (cc0 + 4) * 128, :].rearrange(
                                "(c p) o -> p c o", p=128
                            ),
                        )

                # stats rows (short critical chain)
                negmu_r = rowr.tile([1, 512], f32r, tag="negmu_r")
                nc.vector.tensor_scalar_mul(
                    out=negmu_r, in0=ps_sums, scalar1=-1.0 / HID
                )
                mu = statrow.tile([1, 512], f32, tag="mu")
                nc.vector.tensor_scalar_mul(out=mu, in0=ps_sums, scalar1=1.0 / HID)
                nc.vector.tensor_mul(out=mu, in0=mu, in1=mu)  # mu := mu^2
                var = statrow.tile([1, 512], f32, tag="var")
                nc.vector.scalar_tensor_tensor(
                    out=var,
                    in0=ps_sumsq,
                    scalar=1.0 / HID,
                    in1=mu,
                    op0=mybir.AluOpType.mult,
                    op1=mybir.AluOpType.subtract,
                )
                invrstd_r = rowr.tile([1, 512], f32r, tag="invrstd_r")
                nc.scalar.activation(
                    out=invrstd_r, in_=var, func=Act.Sqrt, bias=eps_tile[0:1]
                )
                rstd = statrow.tile([1, 512], f32, tag="rstd")
                nc.vector.reciprocal_approx_fast(
                    out=rstd, in_=invrstd_r.bitcast(f32)
                )

                # rstd column layout (DRAM bounce) + partition broadcast
                nc.sync.dma_start(out=rstd_dram[sb : sb + 1, :], in_=rstd)
                nc.sync.dma_start(
                    out=rstd_col[:, sb * 4 : (sb + 1) * 4],
                    in_=rstd_dram[sb, :].rearrange("(f p) -> p f", p=128),
                )
                rstd_b = bcastp.tile([128, 512], f32, tag="rstd_b")
                nc.gpsimd.partition_broadcast(rstd_b, rstd)

                # q/k rank-1 corrections + evac (q transient, k persistent)
                qcur = [
                    qpool.tile([128, 512], f32r, tag=f"q{h}", name=f"q{sb}_{h}")
                    for h in range(HPC)
                ]
                for ob in range(4):
                    nc.tensor.matmul(
                        ps_qk[ob],
                        rsum_qk[0:1, ob * 128 : (ob + 1) * 128],
                        negmu_r,
                        start=False,
                        stop=False,
                    )
                    nc.tensor.matmul(
                        ps_qk[ob],
                        bqk[0:1, ob * 128 : (ob + 1) * 128],
                        invrstd_r,
                        start=False,
                        stop=True,
                    )
                    if ob < 2:
                        nc.vector.tensor_mul(
                            out=qcur[ob], in0=ps_qk[ob], in1=rstd_b
                        )
                    else:
                        nc.vector.tensor_mul(
                            out=kT[ob - 2][:, s0 : s0 + 512],
                            in0=ps_qk[ob],
                            in1=rstd_b,
                        )

                # v rank-1 corrections + evac (bf16 out)
                for vs in range(4):
                    pv = ps_v[vs // 2][:, (vs % 2) * 256 : (vs % 2 + 1) * 256]
                    nc.tensor.matmul(
                        pv,
                        negmu_r[0:1, vs * 128 : (vs + 1) * 128],
                        rsum_v,
                        start=False,
                        stop=False,
                        skip_group_check=True,
                    )
                    nc.tensor.matmul(
                        pv,
                        invrstd_r[0:1, vs * 128 : (vs + 1) * 128],
                        bv,
                        start=False,
                        stop=True,
                        skip_group_check=True,
                    )
                    nc.vector.tensor_scalar_mul(
                        out=vtile[:, sb * 4 + vs, :],
                        in0=pv,
                        scalar1=rstd_col[:, sb * 4 + vs : sb * 4 + vs + 1],
                    )

                # ---------------- attention for this sb ----------------
                ntb = 4 * (sb + 1)  # causal t-blocks
                for h in range(HPC):
                    ps_ctx = psp.tile(
                        [128, 512], f32, tag="bank", name=f"ps_ctx{sb}_{h}"
                    )
                    ps_den = psp.tile(
                        [1, 512], f32, tag="bank", name=f"ps_den{sb}_{h}"
                    )
                    for tb in range(ntb):
                        t0 = tb * 128
                        delta = max(0, t0 - s0)
                        ps_sc = psp.tile([128, 512], f32, tag="bank", name="ps_sc")
                        nc.tensor.matmul(
                            ps_sc[:, delta:512],
                            kT[h][:, t0 : t0 + 128],
                            qcur[h][:, delta:512],
                            start=True,
                            stop=(t0 < s0),
                        )
                        if t0 >= s0:
                            nc.tensor.matmul(
                                ps_sc[:, delta : delta + 128],
                                maskneg,
                                ident,
                                start=False,
                                stop=True,
                            )
                        expt = exppool.tile([128, 512], bf16, tag="expt")
                        nc.scalar.activation(
                            out=expt[:, delta:512],
                            in_=ps_sc[:, delta:512],
                            func=Act.Exp,
                            scale=SCALE,
                        )
                        # columns [0, delta) are invalid (t > s) and never
                        # written: every column's first accumulant is tb==0.
                        nc.tensor.matmul(
                            ps_ctx[:, delta:512],
                            vtile[:, tb, h * HD : (h + 1) * HD],
                            expt[:, delta:512],
                            start=(tb == 0),
                            stop=(tb == ntb - 1),
                            skip_group_check=True,
                        )
                        nc.tensor.matmul(
                            ps_den[:, delta:512],
                            ones_bf,
                            expt[:, delta:512],
                            start=(tb == 0),
                            stop=(tb == ntb - 1),
                            skip_group_check=True,
                        )
                    rden = statrow.tile([1, 512], f32, tag="rden")
                    nc.vector.reciprocal_approx_fast(out=rden, in_=ps_den)
                    rden_b = bcastp.tile([128, 512], f32, tag="rden_b")
                    nc.gpsimd.partition_broadcast(rden_b, rden)
                    nc.vector.tensor_mul(
                        out=ctxT[h][:, s0 : s0 + 512], in0=ps_ctx, in1=rden_b
                    )
                    # stage + exchange this head's ctx slices immediately so
                    # the last head's A2A overlaps the other head's attention
                    nc.sync.dma_start(
                        out=ctx_send[sb, h].rearrange("d p r -> p d r"),
                        in_=ctxT[h][:, s0 : s0 + 512].rearrange(
                            "p (d r) -> p d r", d=NCORES
                        ),
                    )
                    if sim_mode:
                        nc.sync.dma_start(
                            out=ctx_recv[sb, h], in_=ctx_send[sb, h]
                        )
                    else:
                        nc.gpsimd.collective_compute(
                            "AllToAll",
                            mybir.AluOpType.bypass,
                            replica_groups=[list(range(NCORES))],
                            ins=[ctx_send[sb, h].rearrange("d p r -> (d p) r")],
                            outs=[ctx_recv[sb, h].rearrange("c p r -> (c p) r")],
                        )

                if sb == 2:
                    # pair-0: its four A2As completed long ago
                    ca0 = emit_pair_loads(0)
                    emit_pair_proj(0, ca0)

            ca1 = emit_pair_loads(1)
            emit_pair_proj(1, ca1, head_major=True)

    nc.finalize()
    return nc


def get_nc(debug=False, sim_mode=False):
    key = ("nc", debug, sim_mode)
    if key not in _CACHE:
        _CACHE[key] = _build_nc(debug=debug, sim_mode=sim_mode)
    return _CACHE[key]


def make_in_maps(hidden_states, ln_weight, ln_bias, qkv_weight, qkv_bias,
                 proj_weight, proj_bias):
    import ml_dtypes

    f4 = np.float32
    bf = ml_dtypes.bfloat16
    x = np.asarray(hidden_states, f4)[:, 0, :]                      # [S, HID]
    xT = np.ascontiguousarray(x.T)                                  # [HID, S]
    g = np.asarray(ln_weight, f4)
    b = np.asarray(ln_bias, f4)
    W = np.asarray(qkv_weight, f4)
    W1 = W * g[None, :]
    b1 = np.asarray(qkv_bias, f4) + W @ b
    W3 = W1.reshape(3, NH, HD, HID)
    b3 = b1.reshape(3, NH, HD)
    pw = np.asarray(proj_weight, f4)
    wpf = np.ascontiguousarray(pw.T).astype(bf)                     # [HID, HID]
    pbias = np.asarray(proj_bias, f4).reshape(1, HID).astype(bf)
    maskneg = np.triu(np.full((128, 128), MASKVAL, f4), 1).astype(bf)
    ident = np.eye(128, dtype=bf)
    ones_col = np.ones((128, 1), f4)
    ones_bf = np.ones((128, 1), bf)
    ones_row_bf = np.ones((1, 128), bf)

    in_maps = []
    for c in range(NCORES):
        hs = slice(HPC * c, HPC * (c + 1))
        Wq = W3[0, hs].reshape(OV, HID)
        Wk = W3[1, hs].reshape(OV, HID)
        Wv = W3[2, hs].reshape(OV, HID)
        Wqk = np.concatenate([Wq, Wk], 0)                           # [512, HID]
        in_maps.append({
            "xT": xT,
            "wqkT": np.ascontiguousarray(Wqk.T),
            "wvT": np.ascontiguousarray(Wv.T),
            "wpf": wpf,
            "rsum_qk": Wqk.sum(1).reshape(1, OQK),
            "rsum_v": Wv.sum(1).reshape(1, OV),
            "bqk": np.concatenate(
                [b3[0, hs].reshape(OV), b3[1, hs].reshape(OV)]
            ).reshape(1, OQK),
            "bv": b3[2, hs].reshape(1, OV),
            "pbias": pbias,
            "maskneg": maskneg,
            "ident": ident,
            "ones_col": ones_col,
            "ones_bf": ones_bf,
            "ones_row_bf": ones_row_bf,
        })
    return in_maps


def assemble(outs):
    """outs: list of per-core [NSB*RS_OUT, HID] arrays -> full [S, 1, HID]."""
    full = np.empty((S, HID), np.float32)
    for c in range(NCORES):
        o = outs[c]
        for sb in range(NSB):
            full[sb * 512 + c * RS_OUT : sb * 512 + (c + 1) * RS_OUT, :] = o[
                sb * RS_OUT : (sb + 1) * RS_OUT, :
            ]
    return full.reshape(S, 1, HID)


class _Runner:
    """Cached PJRT runner: jit once, keep per-core weight shards device-
    resident across calls (re-uploaded only when weight bytes change)."""

    # inputs that depend only on the weights/constants (cacheable on device)
    WEIGHT_NAMES = frozenset({
        "wqkT", "wvT", "wpf", "rsum_qk", "rsum_v", "bqk", "bv", "pbias",
        "maskneg", "ident", "ones_col", "ones_bf", "ones_row_bf",
    })

    def __init__(self, nc):
        import jax
        import concourse.mybir as mybir
        from concourse import bass2jax
        from concourse.bass2jax import _bass_exec_p, partition_id_tensor
        from jax.sharding import Mesh, PartitionSpec
        from jax.experimental.shard_map import shard_map

        bass2jax.install_neuronx_cc_hook()
        self.nc = nc
        self.jax = jax
        partition_name = (
            nc.partition_id_tensor.name if nc.partition_id_tensor else None
        )
        in_names, out_names, out_avals = [], [], []
        for alloc in nc.m.functions[0].allocations:
            if not isinstance(alloc, mybir.MemoryLocationSet):
                continue
            name = alloc.memorylocations[0].name
            if alloc.kind == "ExternalInput":
                if name != partition_name:
                    in_names.append(name)
            elif alloc.kind == "ExternalOutput":
                shape = tuple(alloc.tensor_shape)
                out_names.append(name)
                out_avals.append(
                    jax.core.ShapedArray(shape, mybir.dt.np(alloc.dtype))
                )
        self.in_names, self.out_names, self.out_avals = in_names, out_names, out_avals
        all_in_names = list(in_names) + list(out_names)
        if partition_name is not None:
            all_in_names.append(partition_name)

        def _body(*args):
            operands = list(args)
            if partition_name is not None:
                operands.append(partition_id_tensor())
            return tuple(
                _bass_exec_p.bind(
                    *operands,
                    out_avals=tuple(out_avals),
                    in_names=tuple(all_in_names),
                    out_names=tuple(out_names),
                    lowering_input_output_aliases=(),
                    sim_require_finite=True,
                    sim_require_nnan=True,
                    nc=nc,
                )
            )

        devices = jax.devices()[:NCORES]
        mesh = Mesh(np.asarray(devices), ("core",))
        nin = len(in_names) + len(out_names)
        self._fn = jax.jit(
            shard_map(
                _body,
                mesh=mesh,
                in_specs=(PartitionSpec("core"),) * nin,
                out_specs=(PartitionSpec("core"),) * len(out_names),
                check_rep=False,
            ),
            keep_unused=True,
        )
        self._zeros = [
            np.zeros((NCORES * a.shape[0], *a.shape[1:]), a.dtype)
            for a in out_avals
        ]
        self._weight_cache = {}  # name -> (fingerprint, device_array)
        self._warmed = False

    def __call__(self, in_maps):
        concat = {}
        for i, name in enumerate(self.in_names):
            arr = np.concatenate([np.asarray(m[name]) for m in in_maps], axis=0)
            if name in self.WEIGHT_NAMES:
                fp = hash(arr.tobytes())
                cached = self._weight_cache.get(name)
                if cached is not None and cached[0] == fp:
                    concat[name] = cached[1]
                else:
                    dev = self.jax.device_put(arr)
                    self._weight_cache[name] = (fp, dev)
                    concat[name] = dev
            else:
                concat[name] = arr
        args = [concat[n] for n in self.in_names]
        if not self._warmed:
            # first-ever NEFF execution can race device/semaphore init;
            # run once to warm, take results from the steady-state run
            self.jax.block_until_ready(self._fn(*args, *self._zeros))
            self._warmed = True
        out_arrs = self._fn(*args, *self._zeros)
        outs = []
        for c in range(NCORES):
            outs.append({
                name: np.asarray(out_arrs[i]).reshape(
                    NCORES, *self.out_avals[i].shape
                )[c]
                for i, name in enumerate(self.out_names)
            })
        return outs


def get_runner():
    if "runner" not in _CACHE:
        _CACHE["runner"] = _Runner(get_nc())
    return _CACHE["runner"]


def kernel(hidden_states, ln_weight, ln_bias, qkv_weight, qkv_bias,
           proj_weight, proj_bias):
    in_maps = make_in_maps(hidden_states, ln_weight, ln_bias, qkv_weight,
                           qkv_bias, proj_weight, proj_bias)
    outs = get_runner()(in_maps)
    return assemble([o["out"] for o in outs])
